# revision 4
# baseline (speedup 1.0000x reference)
"""Trainium2 Bass kernel for upsample_conv_2d (conv_transpose stride-2 3x3 +
4x4 FIR + bias), data-parallel over batch on 8 NeuronCores.

Two-stage decomposition (vs the fully-fused 36-phase-tap version):

Stage 1 (PE): conv_transpose with the FIR *column* pass fused into the
weights. Output y'[pa'][r, 2n+beta] = sum_{ci,i,j} x[ci,r+i,n+j]*W2[..]
on the original 64/65-row grid but FINAL 128 columns; 6 taps for
pa'=0, 3 for pa'=1 -> 18 phase-taps total (vs 36 fully fused), so PE
time halves.

Stage 2 (DVE): the FIR *row* pass is a 4-tap vertical mix across the two
row-phase grids z0 (65 rows) and z1 (64 rows, zero halo rows):

    out[2m+0] = k0*z0[m+1] + k2*z0[m] + k1*z1[m]   + k3*z1[m-1]
    out[2m+1] = k1*z0[m+1] + k3*z0[m] + k0*z1[m+1] + k2*z1[m]

k = [.25,.75,.75,.25]. Each parity needs 3 scalar_tensor_tensor passes;
the 1/k3 fold and the bias are absorbed in the ACT PSUM->SBUF drain
(z = k3*psum + k3*bias/2; row taps sum to 2 so bias emerges once; the
two edge rows lose a 0.125*bias term via the zero halo -- ~1e-3 rel,
well inside the 2e-2 gate).

The bench rep-loop runs ON DEVICE via a tile hardware loop (tc.For_i).
"""

import json

import numpy as np

import concourse.bass as bass
import concourse.mybir as mybir
import concourse.tile as tile
from concourse.bass_utils import run_bass_kernel_spmd

# ---------------------------------------------------------------------------
# BIR post-pass: this walrus build rejects instructions carrying more than one
# sem wait (e.g. Tile's kernel-tail Drain gets 3). Hoist extras into
# standalone EventSemaphore instructions right before the owner.
# ---------------------------------------------------------------------------
_MAX_WAITS = 1


def _split_waits(j: dict) -> dict:
    for fn in j.get("functions", []):
        for blk in fn.get("blocks", []):
            insts = blk.get("instructions")
            if not insts:
                continue
            out = []
            for inst in insts:
                si = inst.get("sync_info") or {}
                waits = si.get("on_wait") or []
                if len(waits) > _MAX_WAITS:
                    for k, w in enumerate(waits[_MAX_WAITS:]):
                        out.append(
                            {
                                "debug": inst.get("debug", 0),
                                "engine": inst["engine"],
                                "ins": [],
                                "name": f"{inst['name']}-wsplit{k}",
                                "opcode": "EventSemaphore",
                                "outs": [],
                                "sync_info": {"on_update": [], "on_wait": [w]},
                            }
                        )
                    si["on_wait"] = waits[:_MAX_WAITS]
                out.append(inst)
            blk["instructions"] = out
    return j


_orig_to_json_bytes = bass.Bass.to_json_bytes


def _patched_to_json_bytes(self):
    return json.dumps(_split_waits(json.loads(_orig_to_json_bytes(self)))).encode()


bass.Bass.to_json_bytes = _patched_to_json_bytes

# ---------------------------------------------------------------------------
# Problem constants (hardcoded; kernel.py must be self-contained)
# ---------------------------------------------------------------------------
N, C, H, W = 8, 256, 64, 64
OH, OW = 2 * H, 2 * W
N_CORES = 8
F32 = mybir.dt.float32
BF16 = mybir.dt.bfloat16

K1D = (0.25, 0.75, 0.75, 0.25)  # separable FIR 1D factor (outer sums to 4)

# col-fused weight table: _FWTAB[beta] = [(j, q, k-index), ...]
_FWTAB = (
    ((-1, 0, 2), (-1, 1, 3), (0, 0, 0), (0, 1, 1), (0, 2, 2), (1, 2, 0)),
    ((-1, 0, 3), (0, 0, 1), (0, 1, 2), (0, 2, 3), (1, 1, 0), (1, 2, 1)),
)
# stage-1 row taps per pa': (i, p)
_ROWTAB = (((-1, 0), (0, 2)), ((0, 1),))
# stage-1 tap lists per pa': (i, j) pairs
_TAPS1 = tuple(
    tuple((i, j) for (i, _p) in _ROWTAB[pa] for j in (-1, 0, 1)) for pa in range(2)
)
_COLBASE = (0, len(_TAPS1[0]), 2 * len(_TAPS1[0]), 2 * len(_TAPS1[0]) + len(_TAPS1[1]))
_NCOL = (2 * len(_TAPS1[0]) + 2 * len(_TAPS1[1])) * 4  # (6+6+3+3)*4 = 72


def _col_index(pa: int, beta: int, t: int, cib: int, cob: int) -> int:
    # cob-major so each cob's (and within it, pa=0-first) columns are
    # contiguous -> chunked weight DMA matches first-use order
    return cob * 36 + (_COLBASE[pa * 2 + beta] + t) * 2 + cib


def _stage1_weight_matrix(w: np.ndarray) -> np.ndarray:
    """[256,256,3,3] conv_transpose weight -> [128, 72*128] lhsT matrix."""
    wd = w.astype(np.float64)
    out = np.zeros((128, _NCOL, 128), dtype=np.float32)
    for pa in range(2):
        p_of_i = {iv: pv for (iv, pv) in _ROWTAB[pa]}
        for beta in range(2):
            for t, (i, j) in enumerate(_TAPS1[pa]):
                p = p_of_i[i]
                Kof = np.zeros((C, C), dtype=np.float64)
                for (jv, q, ki) in _FWTAB[beta]:
                    if jv == j:
                        Kof += K1D[ki] * wd[:, :, p, q]
                for cib in range(2):
                    for cob in range(2):
                        cidx = _col_index(pa, beta, t, cib, cob)
                        blk = Kof[
                            cob * 128 : (cob + 1) * 128,
                            cib * 128 : (cib + 1) * 128,
                        ]  # [co, ci]
                        out[:, cidx, :] = blk.T.astype(np.float32)
    return out.reshape(128, -1)


def build_nc(reps: int = 1) -> bass.Bass:
    nc = bass.Bass("TRN2", target_bir_lowering=False, debug=False)
    x_d = nc.dram_tensor("x", [C, H + 2, W + 2], BF16, kind="ExternalInput").ap()
    w_d = nc.dram_tensor("w", [128, _NCOL * 128], BF16, kind="ExternalInput").ap()
    b_d = nc.dram_tensor("bias", [2, 128], F32, kind="ExternalInput").ap()
    out_d = nc.dram_tensor("out", [C, OH, OW], F32, kind="ExternalOutput").ap()

    xb = x_d.rearrange("(b p) h w -> b p h w", p=128)

    c0, c1, c2, c3 = K1D
    mult = mybir.AluOpType.mult
    add = mybir.AluOpType.add

    with tile.TileContext(nc) as tc:
        with (
            tc.tile_pool(name="weights", bufs=1) as wpool,
            tc.tile_pool(name="xin", bufs=1) as xpool,
            tc.tile_pool(name="psum", bufs=8, space="PSUM") as ppool,
            tc.tile_pool(name="zbuf", bufs=2) as zpool,
            tc.tile_pool(name="vtmp", bufs=3) as vpool,
            tc.tile_pool(name="outs", bufs=3) as opool,
        ):
            # split input loads across the two HWDGE queues (SP + ACT)
            xpad = [
                xpool.tile([128, H + 2, W + 2], BF16, tag=f"xp{i}", name=f"xp{i}")
                for i in range(2)
            ]
            xs0 = xb[0].rearrange("p (a h) w -> p a h w", a=3)
            xs1 = xb[1].rearrange("p (a h) w -> p a h w", a=3)
            xt0 = xpad[0][:].rearrange("p (a h) w -> p a h w", a=3)
            xt1 = xpad[1][:].rearrange("p (a h) w -> p a h w", a=3)
            wb = w_d.rearrange("p (a b) -> p a b", b=128)
            wt = wpool.tile([128, _NCOL, 128], BF16)
            # first matmul group (cob0, pa0, chunk0) needs x rows 0..9 of
            # both cibs and wt cols 0..24; stream those first on the two
            # HWDGE queues
            nc.sync.dma_start(xt0[:, 0], xs0[:, 0])
            nc.scalar.dma_start(xt1[:, 0], xs1[:, 0])
            nc.sync.dma_start(wt[:, 0:12], wb[:, 0:12])
            nc.scalar.dma_start(xt1[:, 1], xs1[:, 1])
            nc.sync.dma_start(wt[:, 12:24], wb[:, 12:24])
            nc.sync.dma_start(xt0[:, 1], xs0[:, 1])
            nc.scalar.dma_start(xt1[:, 2], xs1[:, 2])
            nc.sync.dma_start(xt0[:, 2], xs0[:, 2])
            nc.scalar.dma_start(wt[:, 24:_NCOL], wb[:, 24:_NCOL])
            # host pre-scales bias to 3/8*bias
            bt = wpool.tile([128, 2], F32)
            nc.sync.dma_start(bt[:], b_d.rearrange("b p -> p b"))

            def body():
                for cob in range(2):
                    # per-V-block z tiles (17/18 rows incl shared boundary
                    # rows written twice via split drains; z1 halo rows 0)
                    zb0 = [
                        zpool.tile([128, 17, 64, 2], BF16, tag=f"z0{b}", name=f"z0{b}")
                        for b in range(4)
                    ]
                    zb1 = [
                        zpool.tile([128, 18, 64, 2], BF16, tag=f"z1{b}", name=f"z1{b}")
                        for b in range(4)
                    ]
                    nc.vector.memset(zb1[0][:, 0], 0.0)
                    nc.vector.memset(zb1[3][:, 17], 0.0)

                    # ---- stage 1: PE matmuls + split ACT drains,
                    # pa-interleaved row chunks so V blocks unblock early ----
                    for chunk in range(9):
                        for pa in range(2):
                            nrows = 65 if pa == 0 else 64
                            r0 = chunk * 8
                            if r0 >= nrows:
                                continue
                            nr = min(8, nrows - r0)
                            taps = _TAPS1[pa]
                            # taps whose rhs rows fall entirely in the zero
                            # x-padding (row 64's i=0 taps read xpad row 65)
                            # contribute nothing -- drop them
                            eff = [
                                (t, ij)
                                for t, ij in enumerate(taps)
                                if r0 + ij[0] + 1 < 65
                            ]
                            for beta in range(2):
                                ps = ppool.tile(
                                    [128, nr, 64], F32, tag="ps", name="ps"
                                )
                                nmm = len(eff) * 2
                                it = 0
                                for t, (i, j) in eff:
                                    for cib in range(2):
                                        lhsT = wt[
                                            :, _col_index(pa, beta, t, cib, cob), :
                                        ]
                                        rhs = xpad[cib][
                                            :,
                                            r0 + i + 1 : r0 + i + 1 + nr,
                                            j + 1 : j + 1 + 64,
                                        ]
                                        nc.tensor.matmul(
                                            ps[:],
                                            lhsT,
                                            rhs,
                                            start=(it == 0),
                                            stop=(it == nmm - 1),
                                        )
                                        it += 1
                                # split drain into per-V-block z tiles
                                for b in range(4):
                                    if pa == 0:
                                        lo = max(r0, 16 * b)
                                        hi = min(r0 + nr, 16 * b + 17)
                                        off = 16 * b
                                        zt = zb0[b]
                                    else:
                                        lo = max(r0, 16 * b - 1)
                                        hi = min(r0 + nr, 16 * b + 17)
                                        off = 16 * b - 1
                                        zt = zb1[b]
                                    if lo >= hi:
                                        continue
                                    nc.scalar.activation(
                                        zt[:, lo - off : hi - off, :, beta],
                                        ps[:, lo - r0 : hi - r0],
                                        mybir.ActivationFunctionType.Identity,
                                        bias=bt[:, cob : cob + 1],
                                        scale=c2,
                                    )

                    # ---- stage 2: row-FIR on DVE; add/add/stt with the 0.75
                    # folded into the drains. z' = 0.75*y' + 0.375*bias:
                    #   out[2m]   = (z'0[m] + z'1[m])   + (z'0[m+1] + z'1[m-1])/3
                    #   out[2m+1] = (z'0[m+1] + z'1[m]) + (z'0[m] + z'1[m+1])/3
                    # stage is bf16; the output DMA casts to f32 via SWDGE.
                    # The last z block runs as two half-blocks to shorten the
                    # exposed tail after the final matmul.
                    for blk in range(4):
                        z0t, z1t = zb0[blk], zb1[blk]
                        subs = (
                            [(0, 16)]
                            if blk < 3
                            else [(0, 4), (4, 4), (8, 4), (12, 4)]
                        )
                        for s0, sn in subs:
                            stage = opool.tile(
                                [128, sn, 2, 64, 2], BF16,
                                tag=f"st{sn}", name="st",
                            )
                            for alpha in range(2):
                                t1 = vpool.tile(
                                    [128, sn, 64, 2], BF16, tag=f"t1{sn}", name="t1"
                                )
                                t2 = vpool.tile(
                                    [128, sn, 64, 2], BF16, tag=f"t2{sn}", name="t2"
                                )
                                if alpha == 0:
                                    nc.vector.tensor_add(
                                        t1[:], z0t[:, s0 + 1 : s0 + 1 + sn],
                                        z1t[:, s0 : s0 + sn],
                                    )
                                    nc.vector.tensor_add(
                                        t2[:], z0t[:, s0 : s0 + sn],
                                        z1t[:, s0 + 1 : s0 + 1 + sn],
                                    )
                                    nc.vector.scalar_tensor_tensor(
                                        stage[:, :, 0], t1[:], c3 / c2, t2[:],
                                        mult, add,
                                    )
                                else:
                                    nc.vector.tensor_add(
                                        t1[:], z0t[:, s0 : s0 + sn],
                                        z1t[:, s0 + 2 : s0 + 2 + sn],
                                    )
                                    nc.vector.tensor_add(
                                        t2[:], z0t[:, s0 + 1 : s0 + 1 + sn],
                                        z1t[:, s0 + 1 : s0 + 1 + sn],
                                    )
                                    nc.vector.scalar_tensor_tensor(
                                        stage[:, :, 1], t1[:], c3 / c2, t2[:],
                                        mult, add,
                                    )
                            dst = out_d[
                                cob * 128 : (cob + 1) * 128,
                                32 * blk + 2 * s0 : 32 * blk + 2 * s0 + 2 * sn,
                                :,
                            ].rearrange("c (m a) (n b) -> c m a n b", a=2, b=2)
                            nc.gpsimd.dma_start(dst, stage[:])

            if reps == 1:
                body()
            else:
                with tc.For_i(0, reps):
                    body()
    return nc


_CACHED_NC = {}


def _get_nc(reps: int = 1) -> bass.Bass:
    if reps not in _CACHED_NC:
        _CACHED_NC[reps] = build_nc(reps)
    return _CACHED_NC[reps]


def _run(x, weight, bias, reps: int = 1):
    import ml_dtypes

    bf16 = np.dtype(ml_dtypes.bfloat16)
    Wmat = _stage1_weight_matrix(np.asarray(weight, dtype=np.float32)).astype(bf16)
    # fold: drain adds 3/8*bias (row taps scaled by 0.75 sum to 8/3)
    b2 = np.ascontiguousarray(
        np.asarray(bias, dtype=np.float32).reshape(2, 128) * 0.375
    )
    xs = np.pad(
        np.asarray(x, dtype=np.float32), ((0, 0), (0, 0), (1, 1), (1, 1))
    ).astype(bf16)
    nc = _get_nc(reps)
    in_maps = [
        {"x": xs[i], "w": Wmat, "bias": b2} for i in range(N_CORES)
    ]
    res = run_bass_kernel_spmd(nc, in_maps, list(range(N_CORES)))
    return np.stack([res.results[i]["out"] for i in range(N_CORES)])


def kernel(x, weight, bias):
    return _run(x, weight, bias, reps=1)
